# revision 1
# baseline (speedup 1.0000x reference)
"""Sparse 3x3x3 conv (C_in=C_out=1) over N=2M voxels in a 256^3 grid.

Strategy (dense_cnn): densify the sparse voxels into a zero-padded dense
grid laid out [y, x, z] (z innermost), then run the 27-tap stencil as a
2D-banded matmul stencil on the tensor engine.  The partition dim holds
a 12x10 (y,x) *input* tile (out tile 10x8 = 80 rows, +-1 halo
pre-duplicated host-side), so ONE banded 120x128 stationary matrix
covers all 9 (dy,dx) taps; only dz needs 3 PSUM-accumulated passes,
expressed as +-1 element shifts on the contiguous z free dim.  x is
sharded across the 8 cores (32 output x per core = 4 x-tiles of 8).
Each y-tile runs 6 uniform N=512 matmuls into one 2-bank PSUM tile
(4-deep pool), drained by a split scalar/vector copy to fp16 and one
DMA.  Outputs are un-tiled + gathered back to point order on host.
"""

import numpy as np

import concourse.mybir as mybir
import concourse.tile as tile
from concourse import bacc
from concourse.bass_utils import run_bass_kernel_spmd

G = 256
NCORES = 8
XS = G // NCORES       # 32 output x per core
LY, LX = 10, 8         # out tile (y, x)
IY, IX = LY + 2, LX + 2            # 12 x 10 = 120 in-tile partitions
NYT = (G + LY - 1) // LY + 1       # 26 y tiles (260 covers 256+2 pad)
NXT = XS // LX                     # 4 x tiles per core (exact)
YPAD = (NYT - 1) * LY + IY         # 262
ZF = G + 2                         # 258 stored z (z' = z+1)
QOUT = LY * LX                     # 80 useful out partitions

PE_DT = mybir.dt.float16
NP_ST = np.float16


def _build_nc(iters=1):
    nc = bacc.Bacc("TRN2", target_bir_lowering=False, debug=False)
    slab = nc.dram_tensor("slab", [120, NYT, NXT, ZF], PE_DT, kind="ExternalInput")
    wts = nc.dram_tensor("wts", [120, 3, 128], PE_DT, kind="ExternalInput")
    ot = nc.dram_tensor("ot", [QOUT, NYT, NXT, G], PE_DT, kind="ExternalOutput")

    with tile.TileContext(nc) as tc:
        with (
            tc.tile_pool(name="w", bufs=1) as wp,
            tc.tile_pool(name="inp", bufs=NYT) as ip,
            tc.tile_pool(name="ps", bufs=4, space="PSUM") as pp,
            tc.tile_pool(name="ob", bufs=6) as op,
        ):
            wt = wp.tile([120, 3, 128], PE_DT)
            nc.sync.dma_start(out=wt[:], in_=wts[:])

            def body(_i=None):
                for yt in range(NYT):
                    it = ip.tile([120, NXT, ZF], PE_DT, tag="inp", name="it")
                    nc.sync.dma_start(out=it[:], in_=slab[:, yt])
                    ps = pp.tile([128, 2 * 512], mybir.dt.float32,
                                 tag="ps", name="ps")
                    for xtp in range(2):
                        for dzi in range(3):
                            nc.tensor.matmul(
                                ps[:, xtp * 512:(xtp + 1) * 512],
                                wt[:, dzi, :],
                                it[:, xtp * 2:xtp * 2 + 2, dzi:dzi + G],
                                start=(dzi == 0),
                                stop=(dzi == 2),
                                skip_group_check=True,
                            )
                    sb = op.tile([QOUT, 2 * 512], PE_DT, tag="ob", name="sb")
                    # split the PSUM drain across both copy engines
                    nc.scalar.copy(out=sb[:, :512], in_=ps[:QOUT, :512])
                    nc.vector.tensor_copy(sb[:, 512:], ps[:QOUT, 512:])
                    if yt % 2 == 0:
                        nc.gpsimd.dma_start(out=ot[:, yt], in_=sb[:])
                    else:
                        nc.sync.dma_start(out=ot[:, yt], in_=sb[:])

            if iters == 1:
                body()
            else:
                with tc.For_i(0, iters, 1):
                    body()
    nc.finalize()
    return nc


_NC_CACHE = {}


def _get_nc(iters=1, **kw):
    key = (iters, tuple(sorted(kw.items())))
    if key not in _NC_CACHE:
        _NC_CACHE[key] = _build_nc(iters, **kw)
    return _NC_CACHE[key]


def _make_wts(W):
    W27 = np.asarray(W, dtype=np.float32).reshape(27)
    A = np.zeros((120, 3, 128), dtype=np.float32)
    for dzi in range(3):
        for ylo in range(LY):
            for xlo in range(LX):
                q = ylo * LX + xlo
                for dyi in range(3):
                    for dxi in range(3):
                        p = (ylo + dyi) * IX + (xlo + dxi)
                        A[p, dzi, q] = W27[dxi * 9 + dyi * 3 + dzi]
    return A.astype(NP_ST)


def _make_in_maps(coords, feats, W):
    x = coords[:, 0].astype(np.int64)
    y = coords[:, 1].astype(np.int64)
    z = coords[:, 2].astype(np.int64)
    Dp = np.zeros((YPAD, G + 2, ZF), dtype=np.float32)   # [y_pad, x_pad, z_pad]
    # reversed: first occurrence wins on duplicate coords (matches reference)
    Dp[y[::-1] + 1, x[::-1] + 1, z[::-1] + 1] = \
        np.asarray(feats, dtype=np.float32)[::-1, 0]
    Dp = Dp.astype(NP_ST)
    sy, sx, sz = Dp.strides
    wts = _make_wts(W)
    in_maps = []
    for c in range(NCORES):
        Dc = Dp[:, XS * c:XS * c + XS + 2, :]
        t = np.lib.stride_tricks.as_strided(
            Dc, shape=(IY, IX, NYT, NXT, ZF),
            strides=(sy, sx, LY * sy, LX * sx, sz))
        in_maps.append({
            "slab": np.ascontiguousarray(t.reshape(120, NYT, NXT, ZF)),
            "wts": wts,
        })
    return in_maps, x, y, z


def kernel(coords, feats, W):
    coords = np.asarray(coords)
    in_maps, x, y, z = _make_in_maps(coords, feats, W)
    nc = _get_nc(1)
    res = run_bass_kernel_spmd(nc, in_maps, list(range(NCORES)))
    # ot[q, yt, xt, z] -> O[y, x, z]: y = yt*LY + q//LX, x = c*32 + xt*LX + q%LX
    parts = []
    for c in range(NCORES):
        o = res.results[c]["ot"].reshape(LY, LX, NYT, NXT, G)
        o = o.transpose(2, 0, 3, 1, 4).reshape(NYT * LY, NXT * LX, G)
        parts.append(o[:G, :, :])
    Ofull = np.concatenate(parts, axis=1)          # [y, x, z]
    return Ofull[y, x, z].astype(np.float32).reshape(-1, 1)



# revision 2
# speedup vs baseline: 1.1635x; 1.1635x over previous
"""Sparse 3x3x3 conv (C_in=C_out=1) over N=2M voxels in a 256^3 grid.

Strategy (dense_cnn): densify the sparse voxels into a zero-padded dense
grid laid out [y, x, z] (z innermost), then run the 27-tap stencil as a
2D-banded matmul stencil on the tensor engine.  The partition dim holds
a 12x10 (y,x) *input* tile (out tile 10x8 = 80 rows, +-1 halo
pre-duplicated host-side), so ONE banded 120x128 stationary matrix
covers all 9 (dy,dx) taps; only dz needs 3 PSUM-accumulated passes,
expressed as +-1 element shifts on the contiguous z free dim.  x is
sharded across the 8 cores (32 output x per core = 4 x-tiles of 8).

y-tiles are processed in pairs with dz as the OUTER loop within a pair,
so 4 consecutive matmuls share one stationary matrix; a post-scheduling
pass then elides the redundant InstLdweights (the PE array keeps its
weights between matmuls), cutting weight-load overhead ~4x.  Each pair
occupies 4 PSUM banks (2-bank tile per y-tile, 4-deep pool = all 8
banks), drained by a split scalar/vector copy to fp16 and one DMA per
y-tile.  Outputs are un-tiled + gathered back to point order on host.
"""

import numpy as np

import concourse.mybir as mybir
import concourse.tile as tile
from concourse import bacc
from concourse.bass_utils import run_bass_kernel_spmd

G = 256
NCORES = 8
XS = G // NCORES       # 32 output x per core
LY, LX = 10, 8         # out tile (y, x)
IY, IX = LY + 2, LX + 2            # 12 x 10 = 120 in-tile partitions
NYT = (G + LY - 1) // LY + 1       # 26 y tiles (260 covers 256+2 pad)
NXT = XS // LX                     # 4 x tiles per core (exact)
YPAD = (NYT - 1) * LY + IY         # 262
ZF = G + 2                         # 258 stored z (z' = z+1)
QOUT = LY * LX                     # 80 useful out partitions

PE_DT = mybir.dt.float16
NP_ST = np.float16


def _elide_ldweights(nc):
    """Drop InstLdweights whose weights AP equals the PE array state.

    Runs after TileContext exit (scheduled order is final), before
    nc.finalize().  The PE array retains its stationary operand between
    matmuls, so a reload of the identical weights AP is a no-op; the
    paired InstMatmult already carries ldweights=False.  Only sync-free
    loads are dropped (waits/updates stay where the scheduler put them).
    """
    removed = 0
    for blk in nc.m.functions[0].blocks:
        cur = None
        keep = []
        changed = False
        for inst in blk.instructions:
            if isinstance(inst, mybir.InstLdweights):
                ap = inst.ins[0].concise() if inst.ins else None
                si = inst.sync_info
                has_sync = si is not None and (len(si.on_wait) or len(si.on_update))
                if ap is not None and ap == cur and not has_sync:
                    removed += 1
                    changed = True
                    continue
                cur = ap
            keep.append(inst)
        if changed:
            blk.instructions[:] = keep
    return removed


def _build_nc(iters=1, group=2):
    nc = bacc.Bacc("TRN2", target_bir_lowering=False, debug=False)
    slab = nc.dram_tensor("slab", [120, NYT, NXT, ZF], PE_DT, kind="ExternalInput")
    wts = nc.dram_tensor("wts", [120, 3, 128], PE_DT, kind="ExternalInput")
    ot = nc.dram_tensor("ot", [QOUT, NYT, NXT, G], PE_DT, kind="ExternalOutput")

    with tile.TileContext(nc) as tc:
        with (
            tc.tile_pool(name="w", bufs=1) as wp,
            tc.tile_pool(name="inp", bufs=NYT) as ip,
            tc.tile_pool(name="ps", bufs=4, space="PSUM") as pp,
            tc.tile_pool(name="ob", bufs=6) as op,
        ):
            wt = wp.tile([120, 3, 128], PE_DT)
            nc.sync.dma_start(out=wt[:], in_=wts[:])

            def body(_i=None):
                for g0 in range(0, NYT, group):
                    yts = list(range(g0, min(g0 + group, NYT)))
                    its, pss = [], []
                    for y in yts:
                        it = ip.tile([120, NXT, ZF], PE_DT, tag="inp", name="it")
                        nc.sync.dma_start(out=it[:], in_=slab[:, y])
                        its.append(it)
                        pss.append(pp.tile([128, 2 * 512], mybir.dt.float32,
                                           tag="ps", name="ps"))
                    for dzi in range(3):
                        for it, ps in zip(its, pss):
                            for xtp in range(2):
                                nc.tensor.matmul(
                                    ps[:, xtp * 512:(xtp + 1) * 512],
                                    wt[:, dzi, :],
                                    it[:, xtp * 2:xtp * 2 + 2, dzi:dzi + G],
                                    start=(dzi == 0), stop=(dzi == 2),
                                    skip_group_check=True,
                                )
                    for y, ps in zip(yts, pss):
                        sb = op.tile([QOUT, 2 * 512], PE_DT, tag="ob", name="sb")
                        # split the PSUM drain across both copy engines
                        nc.scalar.copy(out=sb[:, :512], in_=ps[:QOUT, :512])
                        nc.vector.tensor_copy(sb[:, 512:], ps[:QOUT, 512:])
                        if y % 2 == 0:
                            nc.gpsimd.dma_start(out=ot[:, y], in_=sb[:])
                        else:
                            nc.sync.dma_start(out=ot[:, y], in_=sb[:])

            if iters == 1:
                body()
            else:
                with tc.For_i(0, iters, 1):
                    body()
    _elide_ldweights(nc)
    nc.finalize()
    return nc


_NC_CACHE = {}


def _get_nc(iters=1, **kw):
    key = (iters, tuple(sorted(kw.items())))
    if key not in _NC_CACHE:
        _NC_CACHE[key] = _build_nc(iters, **kw)
    return _NC_CACHE[key]


def _make_wts(W):
    W27 = np.asarray(W, dtype=np.float32).reshape(27)
    A = np.zeros((120, 3, 128), dtype=np.float32)
    for dzi in range(3):
        for ylo in range(LY):
            for xlo in range(LX):
                q = ylo * LX + xlo
                for dyi in range(3):
                    for dxi in range(3):
                        p = (ylo + dyi) * IX + (xlo + dxi)
                        A[p, dzi, q] = W27[dxi * 9 + dyi * 3 + dzi]
    return A.astype(NP_ST)


def _make_in_maps(coords, feats, W):
    x = coords[:, 0].astype(np.int64)
    y = coords[:, 1].astype(np.int64)
    z = coords[:, 2].astype(np.int64)
    Dp = np.zeros((YPAD, G + 2, ZF), dtype=np.float32)   # [y_pad, x_pad, z_pad]
    # reversed: first occurrence wins on duplicate coords (matches reference)
    Dp[y[::-1] + 1, x[::-1] + 1, z[::-1] + 1] = \
        np.asarray(feats, dtype=np.float32)[::-1, 0]
    Dp = Dp.astype(NP_ST)
    sy, sx, sz = Dp.strides
    wts = _make_wts(W)
    in_maps = []
    for c in range(NCORES):
        Dc = Dp[:, XS * c:XS * c + XS + 2, :]
        t = np.lib.stride_tricks.as_strided(
            Dc, shape=(IY, IX, NYT, NXT, ZF),
            strides=(sy, sx, LY * sy, LX * sx, sz))
        in_maps.append({
            "slab": np.ascontiguousarray(t.reshape(120, NYT, NXT, ZF)),
            "wts": wts,
        })
    return in_maps, x, y, z


def kernel(coords, feats, W):
    coords = np.asarray(coords)
    in_maps, x, y, z = _make_in_maps(coords, feats, W)
    nc = _get_nc(1)
    res = run_bass_kernel_spmd(nc, in_maps, list(range(NCORES)))
    # ot[q, yt, xt, z] -> O[y, x, z]: y = yt*LY + q//LX, x = c*32 + xt*LX + q%LX
    parts = []
    for c in range(NCORES):
        o = res.results[c]["ot"].reshape(LY, LX, NYT, NXT, G)
        o = o.transpose(2, 0, 3, 1, 4).reshape(NYT * LY, NXT * LX, G)
        parts.append(o[:G, :, :])
    Ofull = np.concatenate(parts, axis=1)          # [y, x, z]
    return Ofull[y, x, z].astype(np.float32).reshape(-1, 1)
